# revision 5
# baseline (speedup 1.0000x reference)
"""Trainium2 Bass kernel for nn_LSTMActor: trunk GEMM -> LayerNorm -> Tanh ->
LSTM (16 steps, constant input) -> MLP head -> tanh.

Sharding: data-parallel over batch B=2048 across 8 cores (256 rows each);
all weights replicated. Everything after the trunk runs in a transposed
layout (feature dim on partitions) so no per-step transposes are needed.
"""

import numpy as np
import ml_dtypes

import concourse.bass as bass
import concourse.tile as tile
from concourse import mybir, bacc
from concourse import bass_utils
from concourse.masks import make_identity

BF = ml_dtypes.bfloat16
F32 = mybir.dt.float32
BF16 = mybir.dt.bfloat16

B, R, Fd, H, A, T = 2048, 39200, 1024, 1024, 6, 16
NC_ = 8
BS = B // NC_          # 256 rows per core
NB = BS // 128         # 2 b-tiles per core
KT = 128               # contraction tile
RP = ((R + KT - 1) // KT) * KT   # 39296, padded R
NK = RP // KT          # 307 K-tiles for trunk
KH = H // 128          # 8 K-tiles for H-dim GEMMs
M4 = 4 * H // 128      # 32 M-tiles of gates
H2 = H // 2            # 512
KG = 4                 # trunk K-tiles per DMA batch (512KB wtr + 128KB obsT)

_CACHE = {}


def _build():
    nc = bacc.Bacc("TRN2", target_bir_lowering=False, debug=False)

    obsT_d = nc.dram_tensor("obsT", [RP, BS], BF16, kind="ExternalInput")
    wtr_d = nc.dram_tensor("wtr", [RP, Fd], BF16, kind="ExternalInput")
    wih_d = nc.dram_tensor("wih", [Fd, 4 * H], BF16, kind="ExternalInput")
    whh_d = nc.dram_tensor("whh", [H, 4 * H], BF16, kind="ExternalInput")
    w1_d = nc.dram_tensor("w1", [H, H2], BF16, kind="ExternalInput")
    w2_d = nc.dram_tensor("w2", [H2, A], BF16, kind="ExternalInput")
    btr_d = nc.dram_tensor("btr", [Fd], F32, kind="ExternalInput")
    gam_d = nc.dram_tensor("gam", [Fd], F32, kind="ExternalInput")
    bet_d = nc.dram_tensor("bet", [Fd], F32, kind="ExternalInput")
    bsum_d = nc.dram_tensor("bsum", [4 * H], F32, kind="ExternalInput")
    b1_d = nc.dram_tensor("b1", [H2], F32, kind="ExternalInput")
    b2_d = nc.dram_tensor("b2", [A], F32, kind="ExternalInput")
    mu_d = nc.dram_tensor("mu", [BS, T * A], F32, kind="ExternalOutput")

    AF = mybir.ActivationFunctionType

    def bc(ap1d, p=128):
        return bass.AP(tensor=ap1d.tensor, offset=ap1d.offset,
                       ap=[[0, p]] + [list(x) for x in ap1d.ap])

    with tile.TileContext(nc) as tc:
        with (
            tc.tile_pool(name="const", bufs=1) as cst,
            tc.tile_pool(name="state", bufs=1) as st,
            tc.tile_pool(name="wstream", bufs=2) as ws,
            tc.tile_pool(name="work", bufs=1) as wk,
            tc.tile_pool(name="acts", bufs=2) as ac,
        ):
            # ---- resident constants ----
            ident = cst.tile([128, 128], BF16)
            make_identity(nc, ident)
            whh_sb = cst.tile([128, KH, 4 * H], BF16)     # 64KB/part
            nc.sync.dma_start(whh_sb, whh_d.ap().rearrange("(ko p) n -> p ko n", p=128))
            w1_sb = cst.tile([128, KH, H2], BF16)         # 8KB/part
            nc.sync.dma_start(w1_sb, w1_d.ap().rearrange("(ko p) n -> p ko n", p=128))
            w2_sb = cst.tile([128, H2 // 128, A], BF16)   # [128,4,6]
            nc.sync.dma_start(w2_sb, w2_d.ap().rearrange("(ko p) n -> p ko n", p=128))
            btr_b = cst.tile([128, Fd], F32)
            nc.sync.dma_start(btr_b, bc(btr_d.ap()))
            gam_b = cst.tile([128, Fd], F32)
            nc.sync.dma_start(gam_b, bc(gam_d.ap()))
            bet_b = cst.tile([128, Fd], F32)
            nc.sync.dma_start(bet_b, bc(bet_d.ap()))
            bsum_sb = cst.tile([128, M4], F32)            # [128,32] col m = bsum[m*128+p]
            nc.sync.dma_start(bsum_sb, bsum_d.ap().rearrange("(m p) -> p m", p=128))
            b1_sb = cst.tile([128, H2 // 128], F32)       # [128,4]
            nc.sync.dma_start(b1_sb, b1_d.ap().rearrange("(m p) -> p m", p=128))
            b2_b = cst.tile([128, A], F32)
            nc.sync.dma_start(b2_b, bc(b2_d.ap()))
            eps_t = cst.tile([128, 1], F32)
            nc.vector.memset(eps_t, 1e-5)

            # ---- persistent state ----
            xT = st.tile([128, KH, BS], BF16)             # x^T  [Fd, BS]
            preT = st.tile([128, M4, BS], BF16)            # pre^T [4H, BS] 32KB/part
            c_st = st.tile([128, KH, BS], F32)            # c^T  [H, BS]
            hT = [st.tile([128, KH, BS], BF16, name=f"hT{i}", tag=f"h{i}") for i in range(2)]  # ping-pong
            mu_sb = st.tile([128, NB, T * A], F32)        # [128,2,96]

            wtr_r = wtr_d.ap().rearrange("(ko p) n -> p ko n", p=128)
            obsT_r = obsT_d.ap().rearrange("(ko p) b -> p ko b", p=128)
            wih_r = wih_d.ap().rearrange("(ko p) n -> p ko n", p=128)

            # ================= Phase 1: trunk GEMM =================
            with tc.tile_pool(name="ps_trunk", bufs=1, space="PSUM") as pst:
                psx = pst.tile([128, NB, Fd], F32)        # 8KB/part = 4 banks
                for kg in range(0, NK, KG):
                    kn = min(KG, NK - kg)
                    wt = ws.tile([128, KG, Fd], BF16, tag="wtr")
                    ot = ws.tile([128, KG, BS], BF16, tag="obsT")
                    nc.sync.dma_start(wt[:, :kn, :], wtr_r[:, kg : kg + kn, :])
                    nc.sync.dma_start(ot[:, :kn, :], obsT_r[:, kg : kg + kn, :])
                    for kk in range(kn):
                        k = kg + kk
                        for b in range(NB):
                            lhsT = ot[:, kk, b * 128 : (b + 1) * 128]
                            for n in range(2):
                                nc.tensor.matmul(
                                    psx[:, b, n * 512 : (n + 1) * 512],
                                    lhsT,
                                    wt[:, kk, n * 512 : (n + 1) * 512],
                                    start=(k == 0),
                                    stop=(k == NK - 1),
                                )

                # ============ Phase 2: LayerNorm + tanh ============
                xa = wk.tile([128, NB, Fd], BF16, tag="xa")
                for b in range(NB):
                    xs = wk.tile([128, Fd], F32, tag="xs")
                    nc.vector.tensor_add(xs, psx[:, b, :], btr_b)
                    stats = wk.tile([128, 2, 6], F32, tag="stats")
                    for s in range(2):
                        nc.vector.bn_stats(
                            out=stats[:, s, :], in_=xs[:, s * 512 : (s + 1) * 512]
                        )
                    mv = wk.tile([128, 2], F32, tag="mv")
                    nc.vector.bn_aggr(out=mv, in_=stats)
                    rstd = wk.tile([128, 1], F32, tag="rstd")
                    nc.scalar.activation(
                        out=rstd, in_=mv[:, 1:2], func=AF.Sqrt, bias=eps_t, scale=1.0
                    )
                    nc.vector.reciprocal(out=rstd, in_=rstd)
                    xn = wk.tile([128, Fd], F32, tag="xn")
                    nc.vector.tensor_scalar(
                        out=xn,
                        in0=xs,
                        scalar1=mv[:, 0:1],
                        scalar2=rstd,
                        op0=mybir.AluOpType.subtract,
                        op1=mybir.AluOpType.mult,
                    )
                    nc.vector.tensor_mul(xn, xn, gam_b)
                    nc.vector.tensor_add(xn, xn, bet_b)
                    nc.scalar.activation(out=xa[:, b, :], in_=xn, func=AF.Tanh)

            # ============ Phase 3: transpose x -> xT (bf16) ============
            with tc.tile_pool(name="ps_tr", bufs=4, space="PSUM") as ptr:
                for b in range(NB):
                    for f in range(KH):
                        pt = ptr.tile([128, 128], BF16, tag="tr")
                        nc.tensor.transpose(
                            pt, xa[:, b, f * 128 : (f + 1) * 128], ident
                        )
                        nc.scalar.activation(
                            out=xT[:, f, b * 128 : (b + 1) * 128], in_=pt, func=AF.Copy
                        )

            # ============ Phase 4: pre^T = W_ih^T x^T + bsum ============
            with tc.tile_pool(name="ps_pre", bufs=2, space="PSUM") as ppr:
                for m in range(M4):
                    wm = ws.tile([128, KH, 128], BF16, tag="wih", bufs=3)
                    nc.sync.dma_start(wm, wih_r[:, :, m * 128 : (m + 1) * 128])
                    ps = ppr.tile([128, BS], F32, tag="pre")
                    for k in range(KH):
                        nc.tensor.matmul(
                            ps, wm[:, k, :], xT[:, k, :],
                            start=(k == 0), stop=(k == KH - 1),
                        )
                    nc.vector.tensor_scalar_add(
                        preT[:, m, :], ps, bsum_sb[:, m : m + 1]
                    )

            # ============ Phase 5: LSTM steps ============
            with (
                tc.tile_pool(name="ps_g", bufs=5, space="PSUM") as psg,
                tc.tile_pool(name="ps_m", bufs=2, space="PSUM") as psm,
                tc.tile_pool(name="ps_w2", bufs=1, space="PSUM") as psw,
            ):
                relu1T = st.tile([128, H2 // 128, BS], BF16)

                def cell_update(j, si, sf, tg, so, first):
                    """c[j] = sf*c[j] + si*tg ; h[j] = so*tanh(c[j]) -> h_new."""
                    if first:
                        nc.vector.tensor_mul(c_st[:, j, :], si, tg)
                    else:
                        t1 = ac.tile([128, BS], F32, tag="t1")
                        nc.vector.tensor_mul(t1, si, tg)
                        nc.vector.tensor_mul(c_st[:, j, :], c_st[:, j, :], sf)
                        nc.vector.tensor_add(c_st[:, j, :], c_st[:, j, :], t1)
                    tcn = ac.tile([128, BS], F32, tag="tc")
                    nc.scalar.activation(out=tcn, in_=c_st[:, j, :], func=AF.Tanh)
                    nc.vector.tensor_mul(h_new[:, j, :], so, tcn)

                def mlp_head(t, h_cur):
                    for m in range(H2 // 128):
                        ps = psm.tile([128, BS], F32, tag="m1")
                        for k in range(KH):
                            nc.tensor.matmul(
                                ps, w1_sb[:, k, m * 128 : (m + 1) * 128],
                                h_cur[:, k, :],
                                start=(k == 0), stop=(k == KH - 1),
                            )
                        nc.scalar.activation(
                            out=relu1T[:, m, :], in_=ps, func=AF.Relu,
                            bias=b1_sb[:, m : m + 1], scale=1.0,
                        )
                    for b in range(NB):
                        ps2 = psw.tile([128, A], F32, tag="w2")
                        for k2 in range(H2 // 128):
                            nc.tensor.matmul(
                                ps2,
                                relu1T[:, k2, b * 128 : (b + 1) * 128],
                                w2_sb[:, k2, :],
                                start=(k2 == 0), stop=(k2 == H2 // 128 - 1),
                            )
                        t6 = ac.tile([128, A], F32, tag="t6")
                        nc.vector.tensor_add(t6, ps2, b2_b)
                        nc.scalar.activation(
                            out=mu_sb[:, b, t * A : (t + 1) * A], in_=t6, func=AF.Tanh
                        )

                # ---- step 0: h0 = c0 = 0 -> gates = pre ----
                h_new = hT[0]
                for j in range(KH):
                    si = ac.tile([128, BS], F32, tag="a0")
                    tg = ac.tile([128, BS], F32, tag="a2")
                    so = ac.tile([128, BS], F32, tag="a3")
                    nc.scalar.activation(out=si, in_=preT[:, j, :], func=AF.Sigmoid)
                    nc.scalar.activation(out=tg, in_=preT[:, 16 + j, :], func=AF.Tanh)
                    nc.scalar.activation(out=so, in_=preT[:, 24 + j, :], func=AF.Sigmoid)
                    cell_update(j, si, None, tg, so, first=True)
                mlp_head(0, hT[0])

                # ---- steps 1..15 ----
                for t in range(1, T):
                    h_cur = hT[(t + 1) % 2]
                    h_new = hT[t % 2]
                    for j in range(KH):
                        acts = {}
                        for q in range(4):
                            m = 8 * q + j
                            ps = psg.tile([128, BS], F32, tag="g")
                            for kk in range(KH):
                                k = (kk + j) % KH
                                nc.tensor.matmul(
                                    ps,
                                    whh_sb[:, k, m * 128 : (m + 1) * 128],
                                    h_cur[:, k, :],
                                    start=(kk == 0), stop=(kk == KH - 1),
                                )
                            tmp = ac.tile([128, BS], F32, tag=f"q{q}")
                            nc.vector.tensor_add(tmp, ps, preT[:, m, :])
                            out_a = ac.tile([128, BS], F32, tag=f"a{q}")
                            nc.scalar.activation(
                                out=out_a, in_=tmp,
                                func=AF.Tanh if q == 2 else AF.Sigmoid,
                            )
                            acts[q] = out_a
                        cell_update(j, acts[0], acts[1], acts[2], acts[3], first=False)
                    mlp_head(t, h_new)

            # ---- write out ----
            nc.sync.dma_start(
                mu_d.ap().rearrange("(bt p) f -> p bt f", p=128), mu_sb
            )

    nc.compile()
    return nc


def kernel(**inputs):
    obs = np.asarray(inputs["obs"], np.float32)
    W_trunk = np.asarray(inputs["W_trunk"], np.float32)
    b_trunk = np.asarray(inputs["b_trunk"], np.float32)
    gamma = np.asarray(inputs["gamma"], np.float32)
    beta = np.asarray(inputs["beta"], np.float32)
    W_ih = np.asarray(inputs["W_ih"], np.float32)
    b_ih = np.asarray(inputs["b_ih"], np.float32)
    W_hh = np.asarray(inputs["W_hh"], np.float32)
    b_hh = np.asarray(inputs["b_hh"], np.float32)
    W1 = np.asarray(inputs["W1"], np.float32)
    b1 = np.asarray(inputs["b1"], np.float32)
    W2 = np.asarray(inputs["W2"], np.float32)
    b2 = np.asarray(inputs["b2"], np.float32)
    num_actions = int(np.asarray(inputs["num_actions"]))
    assert num_actions == T, f"kernel hardcodes T={T}, got {num_actions}"
    assert obs.shape == (B, R)

    if "nc" not in _CACHE:
        _CACHE["nc"] = _build()
    nc = _CACHE["nc"]

    wtr = np.zeros((RP, Fd), BF)
    wtr[:R] = W_trunk.astype(BF)
    wih = W_ih.astype(BF)
    whh = W_hh.astype(BF)
    w1 = W1.astype(BF)
    w2 = W2.astype(BF)
    bsum = (b_ih + b_hh).astype(np.float32)

    in_maps = []
    for i in range(NC_):
        sh = obs[i * BS : (i + 1) * BS]           # [256, R]
        obsT = np.zeros((RP, BS), BF)
        obsT[:R] = np.ascontiguousarray(sh.T).astype(BF)
        in_maps.append({
            "obsT": obsT, "wtr": wtr, "wih": wih, "whh": whh,
            "w1": w1, "w2": w2, "btr": b_trunk, "gam": gamma,
            "bet": beta, "bsum": bsum, "b1": b1, "b2": b2,
        })

    res = bass_utils.run_bass_kernel_spmd(
        nc, in_maps, core_ids=list(range(NC_)),
        trace=bool(int(__import__("os").environ.get("KTRACE", "0"))),
    )
    _CACHE["last_result"] = res
    out = np.concatenate(
        [res.results[i]["mu"].reshape(BS, T, A) for i in range(NC_)], axis=0
    )
    return out


# revision 6
# speedup vs baseline: 1.2873x; 1.2873x over previous
"""Trainium2 Bass kernel for nn_LSTMActor: trunk GEMM -> LayerNorm -> Tanh ->
LSTM (16 steps, constant input) -> MLP head -> tanh.

Sharding: data-parallel over batch B=2048 across 8 cores (256 rows each);
all weights replicated. Everything after the trunk runs in a transposed
layout (feature dim on partitions) so no per-step transposes are needed.
"""

import numpy as np
import ml_dtypes

import concourse.bass as bass
import concourse.tile as tile
from concourse import mybir, bacc
from concourse import bass_utils
from concourse.masks import make_identity

BF = ml_dtypes.bfloat16
F32 = mybir.dt.float32
BF16 = mybir.dt.bfloat16

B, R, Fd, H, A, T = 2048, 39200, 1024, 1024, 6, 16
NC_ = 8
BS = B // NC_          # 256 rows per core
NB = BS // 128         # 2 b-tiles per core
KT = 128               # contraction tile
RP = ((R + KT - 1) // KT) * KT   # 39296, padded R
NK = RP // KT          # 307 K-tiles for trunk
KH = H // 128          # 8 K-tiles for H-dim GEMMs
M4 = 4 * H // 128      # 32 M-tiles of gates
H2 = H // 2            # 512
KG = 2                 # trunk K-tiles per DMA batch (256KB wtr + 64KB obsT)

_CACHE = {}


def _build():
    nc = bacc.Bacc("TRN2", target_bir_lowering=False, debug=False)

    obsT_d = nc.dram_tensor("obsT", [RP, BS], BF16, kind="ExternalInput")
    wtr_d = nc.dram_tensor("wtr", [RP, Fd], BF16, kind="ExternalInput")
    wih_d = nc.dram_tensor("wih", [Fd, 4 * H], BF16, kind="ExternalInput")
    whh_d = nc.dram_tensor("whh", [H, 4 * H], BF16, kind="ExternalInput")
    w1_d = nc.dram_tensor("w1", [H, H2], BF16, kind="ExternalInput")
    w2_d = nc.dram_tensor("w2", [H2, A], BF16, kind="ExternalInput")
    btr_d = nc.dram_tensor("btr", [Fd], F32, kind="ExternalInput")
    gam_d = nc.dram_tensor("gam", [Fd], F32, kind="ExternalInput")
    bet_d = nc.dram_tensor("bet", [Fd], F32, kind="ExternalInput")
    bsum_d = nc.dram_tensor("bsum", [4 * H], F32, kind="ExternalInput")
    b1_d = nc.dram_tensor("b1", [H2], F32, kind="ExternalInput")
    b2_d = nc.dram_tensor("b2", [A], F32, kind="ExternalInput")
    mu_d = nc.dram_tensor("mu", [BS, T * A], F32, kind="ExternalOutput")

    AF = mybir.ActivationFunctionType

    def bc(ap1d, p=128):
        return bass.AP(tensor=ap1d.tensor, offset=ap1d.offset,
                       ap=[[0, p]] + [list(x) for x in ap1d.ap])

    with tile.TileContext(nc) as tc:
        with (
            tc.tile_pool(name="const", bufs=1) as cst,
            tc.tile_pool(name="state", bufs=1) as st,
            tc.tile_pool(name="wstream", bufs=2) as ws,
            tc.tile_pool(name="work", bufs=1) as wk,
            tc.tile_pool(name="acts", bufs=2) as ac,
        ):
            # ---- resident constants ----
            ident = cst.tile([128, 128], BF16)
            make_identity(nc, ident)
            whh_sb = cst.tile([128, KH, 4 * H], BF16)     # 64KB/part
            w1_sb = cst.tile([128, KH, H2], BF16)         # 8KB/part
            w2_sb = cst.tile([128, H2 // 128, A], BF16)   # [128,4,6]
            btr_b = cst.tile([128, Fd], F32)
            nc.sync.dma_start(btr_b, bc(btr_d.ap()))
            gam_b = cst.tile([128, Fd], F32)
            nc.sync.dma_start(gam_b, bc(gam_d.ap()))
            bet_b = cst.tile([128, Fd], F32)
            nc.sync.dma_start(bet_b, bc(bet_d.ap()))
            bsum_sb = cst.tile([128, M4], F32)            # [128,32] col m = bsum[m*128+p]
            nc.sync.dma_start(bsum_sb, bsum_d.ap().rearrange("(m p) -> p m", p=128))
            b1_sb = cst.tile([128, H2 // 128], F32)       # [128,4]
            nc.sync.dma_start(b1_sb, b1_d.ap().rearrange("(m p) -> p m", p=128))
            b2_b = cst.tile([128, A], F32)
            nc.sync.dma_start(b2_b, bc(b2_d.ap()))
            eps_t = cst.tile([128, 1], F32)
            nc.vector.memset(eps_t, 1e-5)

            # ---- persistent state ----
            xT = st.tile([128, KH, BS], BF16)             # x^T  [Fd, BS]
            preT = st.tile([128, M4, BS], BF16)            # pre^T [4H, BS] 32KB/part
            c_st = st.tile([128, KH, BS], F32)            # c^T  [H, BS]
            hT = [st.tile([128, KH, BS], BF16, name=f"hT{i}", tag=f"h{i}") for i in range(2)]  # ping-pong
            mu_sb = st.tile([128, NB, T * A], F32)        # [128,2,96]

            wtr_r = wtr_d.ap().rearrange("(ko p) n -> p ko n", p=128)
            obsT_r = obsT_d.ap().rearrange("(ko p) b -> p ko b", p=128)
            wih_r = wih_d.ap().rearrange("(ko p) n -> p ko n", p=128)

            # ================= Phase 1: trunk GEMM =================
            with tc.tile_pool(name="ps_trunk", bufs=1, space="PSUM") as pst:
                psx = pst.tile([128, NB, Fd], F32)        # 8KB/part = 4 banks
                for kg in range(0, NK, KG):
                    kn = min(KG, NK - kg)
                    wt = ws.tile([128, KG, Fd], BF16, tag="wtr", bufs=4)
                    ot = ws.tile([128, KG, BS], BF16, tag="obsT", bufs=4)
                    nc.sync.dma_start(wt[:, :kn, :], wtr_r[:, kg : kg + kn, :])
                    nc.sync.dma_start(ot[:, :kn, :], obsT_r[:, kg : kg + kn, :])
                    for kk in range(kn):
                        k = kg + kk
                        for b in range(NB):
                            lhsT = ot[:, kk, b * 128 : (b + 1) * 128]
                            for n in range(2):
                                nc.tensor.matmul(
                                    psx[:, b, n * 512 : (n + 1) * 512],
                                    lhsT,
                                    wt[:, kk, n * 512 : (n + 1) * 512],
                                    start=(k == 0),
                                    stop=(k == NK - 1),
                                )

                # big resident weights: emitted after trunk so their DMA
                # overlaps trunk compute instead of delaying its first chunks
                nc.sync.dma_start(whh_sb, whh_d.ap().rearrange("(ko p) n -> p ko n", p=128))
                nc.sync.dma_start(w1_sb, w1_d.ap().rearrange("(ko p) n -> p ko n", p=128))
                nc.sync.dma_start(w2_sb, w2_d.ap().rearrange("(ko p) n -> p ko n", p=128))

                # ============ Phase 2: LayerNorm + tanh ============
                xa = wk.tile([128, NB, Fd], BF16, tag="xa")
                for b in range(NB):
                    xs = wk.tile([128, Fd], F32, tag="xs", bufs=2)
                    nc.vector.tensor_add(xs, psx[:, b, :], btr_b)
                    stats = wk.tile([128, 2, 6], F32, tag="stats")
                    for s in range(2):
                        nc.vector.bn_stats(
                            out=stats[:, s, :], in_=xs[:, s * 512 : (s + 1) * 512]
                        )
                    mv = wk.tile([128, 2], F32, tag="mv")
                    nc.vector.bn_aggr(out=mv, in_=stats)
                    rstd = wk.tile([128, 1], F32, tag="rstd")
                    nc.scalar.activation(
                        out=rstd, in_=mv[:, 1:2], func=AF.Sqrt, bias=eps_t, scale=1.0
                    )
                    nc.vector.reciprocal(out=rstd, in_=rstd)
                    nc.vector.scalar_tensor_tensor(
                        out=xs, in0=xs, scalar=mv[:, 0:1], in1=gam_b,
                        op0=mybir.AluOpType.subtract, op1=mybir.AluOpType.mult,
                    )
                    nc.vector.scalar_tensor_tensor(
                        out=xs, in0=xs, scalar=rstd, in1=bet_b,
                        op0=mybir.AluOpType.mult, op1=mybir.AluOpType.add,
                    )
                    nc.scalar.activation(out=xa[:, b, :], in_=xs, func=AF.Tanh)

            # ============ Phase 3: transpose x -> xT (bf16) ============
            with tc.tile_pool(name="ps_tr", bufs=4, space="PSUM") as ptr:
                for b in range(NB):
                    for f in range(KH):
                        pt = ptr.tile([128, 128], BF16, tag="tr")
                        nc.tensor.transpose(
                            pt, xa[:, b, f * 128 : (f + 1) * 128], ident
                        )
                        nc.scalar.activation(
                            out=xT[:, f, b * 128 : (b + 1) * 128], in_=pt, func=AF.Copy
                        )

            # ============ Phase 4: pre^T = W_ih^T x^T + bsum ============
            with tc.tile_pool(name="ps_pre", bufs=2, space="PSUM") as ppr:
                for m in range(M4):
                    wm = ws.tile([128, KH, 128], BF16, tag="wih", bufs=3)
                    nc.sync.dma_start(wm, wih_r[:, :, m * 128 : (m + 1) * 128])
                    ps = ppr.tile([128, BS], F32, tag="pre")
                    for k in range(KH):
                        nc.tensor.matmul(
                            ps, wm[:, k, :], xT[:, k, :],
                            start=(k == 0), stop=(k == KH - 1),
                        )
                    nc.vector.tensor_scalar_add(
                        preT[:, m, :], ps, bsum_sb[:, m : m + 1]
                    )

            # ============ Phase 5: LSTM steps ============
            with (
                tc.tile_pool(name="ps_g", bufs=5, space="PSUM") as psg,
                tc.tile_pool(name="ps_m", bufs=2, space="PSUM") as psm,
                tc.tile_pool(name="ps_w2", bufs=1, space="PSUM") as psw,
            ):
                relu1T = st.tile([128, H2 // 128, BS], BF16)

                def cell_update(j, si, sf, tg, so, first):
                    """c[j] = sf*c[j] + si*tg ; h[j] = so*tanh(c[j]) -> h_new."""
                    if first:
                        nc.vector.tensor_mul(c_st[:, j, :], si, tg)
                    else:
                        t1 = ac.tile([128, BS], F32, tag="t1")
                        nc.vector.tensor_mul(t1, si, tg)
                        nc.vector.tensor_mul(c_st[:, j, :], c_st[:, j, :], sf)
                        nc.vector.tensor_add(c_st[:, j, :], c_st[:, j, :], t1)
                    tcn = ac.tile([128, BS], F32, tag="tc")
                    nc.scalar.activation(out=tcn, in_=c_st[:, j, :], func=AF.Tanh)
                    nc.vector.tensor_mul(h_new[:, j, :], so, tcn)

                def mlp_head(t, h_cur):
                    for m in range(H2 // 128):
                        ps = psm.tile([128, BS], F32, tag="m1")
                        for k in range(KH):
                            nc.tensor.matmul(
                                ps, w1_sb[:, k, m * 128 : (m + 1) * 128],
                                h_cur[:, k, :],
                                start=(k == 0), stop=(k == KH - 1),
                            )
                        nc.scalar.activation(
                            out=relu1T[:, m, :], in_=ps, func=AF.Relu,
                            bias=b1_sb[:, m : m + 1], scale=1.0,
                        )
                    for b in range(NB):
                        ps2 = psw.tile([128, A], F32, tag="w2")
                        for k2 in range(H2 // 128):
                            nc.tensor.matmul(
                                ps2,
                                relu1T[:, k2, b * 128 : (b + 1) * 128],
                                w2_sb[:, k2, :],
                                start=(k2 == 0), stop=(k2 == H2 // 128 - 1),
                            )
                        t6 = ac.tile([128, A], F32, tag="t6")
                        nc.vector.tensor_add(t6, ps2, b2_b)
                        nc.scalar.activation(
                            out=mu_sb[:, b, t * A : (t + 1) * A], in_=t6, func=AF.Tanh
                        )

                # ---- step 0: h0 = c0 = 0 -> gates = pre ----
                h_new = hT[0]
                for j in range(KH):
                    si = ac.tile([128, BS], F32, tag="a0")
                    tg = ac.tile([128, BS], F32, tag="a2")
                    so = ac.tile([128, BS], F32, tag="a3")
                    nc.scalar.activation(out=si, in_=preT[:, j, :], func=AF.Sigmoid)
                    nc.scalar.activation(out=tg, in_=preT[:, 16 + j, :], func=AF.Tanh)
                    nc.scalar.activation(out=so, in_=preT[:, 24 + j, :], func=AF.Sigmoid)
                    cell_update(j, si, None, tg, so, first=True)
                mlp_head(0, hT[0])

                # ---- steps 1..15 ----
                for t in range(1, T):
                    h_cur = hT[(t + 1) % 2]
                    h_new = hT[t % 2]
                    for j in range(KH):
                        acts = {}
                        for q in range(4):
                            m = 8 * q + j
                            ps = psg.tile([128, BS], F32, tag="g")
                            for kk in range(KH):
                                k = (kk + j) % KH
                                nc.tensor.matmul(
                                    ps,
                                    whh_sb[:, k, m * 128 : (m + 1) * 128],
                                    h_cur[:, k, :],
                                    start=(kk == 0), stop=(kk == KH - 1),
                                )
                            tmp = ac.tile([128, BS], F32, tag=f"q{q}")
                            nc.vector.tensor_add(tmp, ps, preT[:, m, :])
                            out_a = ac.tile([128, BS], F32, tag=f"a{q}")
                            nc.scalar.activation(
                                out=out_a, in_=tmp,
                                func=AF.Tanh if q == 2 else AF.Sigmoid,
                            )
                            acts[q] = out_a
                        cell_update(j, acts[0], acts[1], acts[2], acts[3], first=False)
                    mlp_head(t, h_new)

            # ---- write out ----
            nc.sync.dma_start(
                mu_d.ap().rearrange("(bt p) f -> p bt f", p=128), mu_sb
            )

    nc.compile()
    return nc


def kernel(**inputs):
    obs = np.asarray(inputs["obs"], np.float32)
    W_trunk = np.asarray(inputs["W_trunk"], np.float32)
    b_trunk = np.asarray(inputs["b_trunk"], np.float32)
    gamma = np.asarray(inputs["gamma"], np.float32)
    beta = np.asarray(inputs["beta"], np.float32)
    W_ih = np.asarray(inputs["W_ih"], np.float32)
    b_ih = np.asarray(inputs["b_ih"], np.float32)
    W_hh = np.asarray(inputs["W_hh"], np.float32)
    b_hh = np.asarray(inputs["b_hh"], np.float32)
    W1 = np.asarray(inputs["W1"], np.float32)
    b1 = np.asarray(inputs["b1"], np.float32)
    W2 = np.asarray(inputs["W2"], np.float32)
    b2 = np.asarray(inputs["b2"], np.float32)
    num_actions = int(np.asarray(inputs["num_actions"]))
    assert num_actions == T, f"kernel hardcodes T={T}, got {num_actions}"
    assert obs.shape == (B, R)

    if "nc" not in _CACHE:
        _CACHE["nc"] = _build()
    nc = _CACHE["nc"]

    wtr = np.zeros((RP, Fd), BF)
    wtr[:R] = W_trunk.astype(BF)
    wih = W_ih.astype(BF)
    whh = W_hh.astype(BF)
    w1 = W1.astype(BF)
    w2 = W2.astype(BF)
    bsum = (b_ih + b_hh).astype(np.float32)

    in_maps = []
    for i in range(NC_):
        sh = obs[i * BS : (i + 1) * BS]           # [256, R]
        obsT = np.zeros((RP, BS), BF)
        obsT[:R] = np.ascontiguousarray(sh.T).astype(BF)
        in_maps.append({
            "obsT": obsT, "wtr": wtr, "wih": wih, "whh": whh,
            "w1": w1, "w2": w2, "btr": b_trunk, "gam": gamma,
            "bet": beta, "bsum": bsum, "b1": b1, "b2": b2,
        })

    res = bass_utils.run_bass_kernel_spmd(
        nc, in_maps, core_ids=list(range(NC_)),
        trace=bool(int(__import__("os").environ.get("KTRACE", "0"))),
    )
    _CACHE["last_result"] = res
    out = np.concatenate(
        [res.results[i]["mu"].reshape(BS, T, A) for i in range(NC_)], axis=0
    )
    return out


# revision 7
# speedup vs baseline: 1.3774x; 1.0700x over previous
"""Trainium2 Bass kernel for nn_LSTMActor: trunk GEMM -> LayerNorm -> Tanh ->
LSTM (16 steps, constant input) -> MLP head -> tanh.

Sharding: data-parallel over batch B=2048 across 8 cores (256 rows each);
all weights replicated. Everything after the trunk runs in a transposed
layout (feature dim on partitions) so no per-step transposes are needed.
"""

import numpy as np
import ml_dtypes

import concourse.bass as bass
import concourse.tile as tile
from concourse import mybir, bacc
from concourse import bass_utils
from concourse.masks import make_identity

BF = ml_dtypes.bfloat16
F32 = mybir.dt.float32
BF16 = mybir.dt.bfloat16

B, R, Fd, H, A, T = 2048, 39200, 1024, 1024, 6, 16
NC_ = 8
BS = B // NC_          # 256 rows per core
NB = BS // 128         # 2 b-tiles per core
KT = 128               # contraction tile
RP = ((R + KT - 1) // KT) * KT   # 39296, padded R
NK = RP // KT          # 307 K-tiles for trunk
KH = H // 128          # 8 K-tiles for H-dim GEMMs
M4 = 4 * H // 128      # 32 M-tiles of gates
H2 = H // 2            # 512
KG = 2                 # trunk K-tiles per DMA batch (256KB wtr + 64KB obsT)

_CACHE = {}


def _build():
    nc = bacc.Bacc("TRN2", target_bir_lowering=False, debug=False)

    obsT_d = nc.dram_tensor("obsT", [RP, BS], BF16, kind="ExternalInput")
    wtr_d = nc.dram_tensor("wtr", [RP, Fd], BF16, kind="ExternalInput")
    wih_d = nc.dram_tensor("wih", [Fd, 4 * H], BF16, kind="ExternalInput")
    whh_d = nc.dram_tensor("whh", [H, 4 * H], BF16, kind="ExternalInput")
    w1_d = nc.dram_tensor("w1", [H, H2], BF16, kind="ExternalInput")
    w2_d = nc.dram_tensor("w2", [H2, A], BF16, kind="ExternalInput")
    btr_d = nc.dram_tensor("btr", [Fd], F32, kind="ExternalInput")
    gam_d = nc.dram_tensor("gam", [Fd], F32, kind="ExternalInput")
    bet_d = nc.dram_tensor("bet", [Fd], F32, kind="ExternalInput")
    bsum_d = nc.dram_tensor("bsum", [4 * H], F32, kind="ExternalInput")
    b1_d = nc.dram_tensor("b1", [H2], F32, kind="ExternalInput")
    b2_d = nc.dram_tensor("b2", [A], F32, kind="ExternalInput")
    mu_d = nc.dram_tensor("mu", [BS, T * A], F32, kind="ExternalOutput")

    AF = mybir.ActivationFunctionType

    def bc(ap1d, p=128):
        return bass.AP(tensor=ap1d.tensor, offset=ap1d.offset,
                       ap=[[0, p]] + [list(x) for x in ap1d.ap])

    with tile.TileContext(nc) as tc:
        with (
            tc.tile_pool(name="const", bufs=1) as cst,
            tc.tile_pool(name="state", bufs=1) as st,
            tc.tile_pool(name="wstream", bufs=2) as ws,
            tc.tile_pool(name="work", bufs=1) as wk,
            tc.tile_pool(name="acts", bufs=2) as ac,
        ):
            # ---- resident constants ----
            ident = cst.tile([128, 128], BF16)
            make_identity(nc, ident)
            whh_sb = cst.tile([128, KH, 4 * H], BF16)     # 64KB/part
            w1_sb = cst.tile([128, KH, H2], BF16)         # 8KB/part
            w2_sb = cst.tile([128, H2 // 128, A], BF16)   # [128,4,6]
            btr_b = cst.tile([128, Fd], F32)
            gam_b = cst.tile([128, Fd], F32)
            bet_b = cst.tile([128, Fd], F32)
            bsum_sb = cst.tile([128, M4], F32)            # [128,32] col m = bsum[m*128+p]
            nc.sync.dma_start(bsum_sb, bsum_d.ap().rearrange("(m p) -> p m", p=128))
            b1_sb = cst.tile([128, H2 // 128], F32)       # [128,4]
            nc.sync.dma_start(b1_sb, b1_d.ap().rearrange("(m p) -> p m", p=128))
            b2_b = cst.tile([128, A], F32)
            nc.sync.dma_start(b2_b, bc(b2_d.ap()))
            eps_t = cst.tile([128, 1], F32)
            nc.vector.memset(eps_t, 1e-5)

            # ---- persistent state ----
            xT = st.tile([128, KH, BS], BF16)             # x^T  [Fd, BS]
            preT = st.tile([128, M4, BS], BF16)            # pre^T [4H, BS] 32KB/part
            c_st = st.tile([128, KH, BS], F32)            # c^T  [H, BS]
            hT = [st.tile([128, KH, BS], BF16, name=f"hT{i}", tag=f"h{i}") for i in range(2)]  # ping-pong
            mu_sb = st.tile([128, NB, T * A], F32)        # [128,2,96]

            wtr_r = wtr_d.ap().rearrange("(ko p) n -> p ko n", p=128)
            obsT_r = obsT_d.ap().rearrange("(ko p) b -> p ko b", p=128)
            wih_r = wih_d.ap().rearrange("(ko p) n -> p ko n", p=128)

            # ================= Phase 1: trunk GEMM =================
            with tc.tile_pool(name="ps_trunk", bufs=1, space="PSUM") as pst:
                psx = pst.tile([128, NB, Fd], F32)        # 8KB/part = 4 banks
                for kg in range(0, NK, KG):
                    kn = min(KG, NK - kg)
                    wt = ws.tile([128, KG, Fd], BF16, tag="wtr", bufs=6)
                    ot = ws.tile([128, KG, BS], BF16, tag="obsT", bufs=4)
                    nc.sync.dma_start(wt[:, :kn, :], wtr_r[:, kg : kg + kn, :])
                    nc.sync.dma_start(ot[:, :kn, :], obsT_r[:, kg : kg + kn, :])
                    for kk in range(kn):
                        k = kg + kk
                        for b in range(NB):
                            lhsT = ot[:, kk, b * 128 : (b + 1) * 128]
                            for n in range(2):
                                nc.tensor.matmul(
                                    psx[:, b, n * 512 : (n + 1) * 512],
                                    lhsT,
                                    wt[:, kk, n * 512 : (n + 1) * 512],
                                    start=(k == 0),
                                    stop=(k == NK - 1),
                                )

                # LN constants: needed right after trunk; emitted here so the
                # trunk's first chunks aren't queued behind them
                nc.sync.dma_start(btr_b, bc(btr_d.ap()))
                nc.sync.dma_start(gam_b, bc(gam_d.ap()))
                nc.sync.dma_start(bet_b, bc(bet_d.ap()))

                # ============ Phase 2: LayerNorm + tanh ============
                xa = wk.tile([128, NB, Fd], BF16, tag="xa")
                for b in range(NB):
                    xs = wk.tile([128, Fd], F32, tag="xs", bufs=2)
                    nc.vector.tensor_add(xs, psx[:, b, :], btr_b)
                    stats = wk.tile([128, 2, 6], F32, tag="stats")
                    for s in range(2):
                        nc.vector.bn_stats(
                            out=stats[:, s, :], in_=xs[:, s * 512 : (s + 1) * 512]
                        )
                    mv = wk.tile([128, 2], F32, tag="mv")
                    nc.vector.bn_aggr(out=mv, in_=stats)
                    rstd = wk.tile([128, 1], F32, tag="rstd")
                    nc.scalar.activation(
                        out=rstd, in_=mv[:, 1:2], func=AF.Sqrt, bias=eps_t, scale=1.0
                    )
                    nc.vector.reciprocal(out=rstd, in_=rstd)
                    nc.vector.scalar_tensor_tensor(
                        out=xs, in0=xs, scalar=mv[:, 0:1], in1=gam_b,
                        op0=mybir.AluOpType.subtract, op1=mybir.AluOpType.mult,
                    )
                    nc.vector.scalar_tensor_tensor(
                        out=xs, in0=xs, scalar=rstd, in1=bet_b,
                        op0=mybir.AluOpType.mult, op1=mybir.AluOpType.add,
                    )
                    nc.scalar.activation(out=xa[:, b, :], in_=xs, func=AF.Tanh)

            # ============ Phase 3: transpose x -> xT (bf16) ============
            with tc.tile_pool(name="ps_tr", bufs=4, space="PSUM") as ptr:
                for b in range(NB):
                    for f in range(KH):
                        pt = ptr.tile([128, 128], BF16, tag="tr")
                        nc.tensor.transpose(
                            pt, xa[:, b, f * 128 : (f + 1) * 128], ident
                        )
                        nc.scalar.activation(
                            out=xT[:, f, b * 128 : (b + 1) * 128], in_=pt, func=AF.Copy
                        )

            # ============ Phase 4: pre^T = W_ih^T x^T + bsum ============
            with tc.tile_pool(name="ps_pre", bufs=2, space="PSUM") as ppr:
                whh_r = whh_d.ap().rearrange("(ko p) n -> p ko n", p=128)
                for m in range(M4):
                    wm = ws.tile([128, KH, 128], BF16, tag="wih", bufs=3)
                    nc.sync.dma_start(wm, wih_r[:, :, m * 128 : (m + 1) * 128])
                    if m % 4 == 0:
                        k8 = m // 4
                        nc.sync.dma_start(whh_sb[:, k8, :], whh_r[:, k8, :])
                        if k8 == 0:
                            nc.sync.dma_start(
                                w1_sb, w1_d.ap().rearrange("(ko p) n -> p ko n", p=128))
                            nc.sync.dma_start(
                                w2_sb, w2_d.ap().rearrange("(ko p) n -> p ko n", p=128))
                    ps = ppr.tile([128, BS], F32, tag="pre")
                    for k in range(KH):
                        nc.tensor.matmul(
                            ps, wm[:, k, :], xT[:, k, :],
                            start=(k == 0), stop=(k == KH - 1),
                        )
                    nc.vector.tensor_scalar_add(
                        preT[:, m, :], ps, bsum_sb[:, m : m + 1]
                    )

            # ============ Phase 5: LSTM steps ============
            with (
                tc.tile_pool(name="ps_g", bufs=5, space="PSUM") as psg,
                tc.tile_pool(name="ps_m", bufs=2, space="PSUM") as psm,
                tc.tile_pool(name="ps_w2", bufs=1, space="PSUM") as psw,
            ):
                relu1T = st.tile([128, H2 // 128, BS], BF16)

                def cell_update(j, si, sf, tg, so, first):
                    """c[j] = sf*c[j] + si*tg ; h[j] = so*tanh(c[j]) -> h_new."""
                    if first:
                        nc.vector.tensor_mul(c_st[:, j, :], si, tg)
                    else:
                        t1 = ac.tile([128, BS], F32, tag="t1")
                        nc.vector.tensor_mul(t1, si, tg)
                        nc.vector.tensor_mul(c_st[:, j, :], c_st[:, j, :], sf)
                        nc.vector.tensor_add(c_st[:, j, :], c_st[:, j, :], t1)
                    tcn = ac.tile([128, BS], F32, tag="tc")
                    nc.scalar.activation(out=tcn, in_=c_st[:, j, :], func=AF.Tanh)
                    nc.vector.tensor_mul(h_new[:, j, :], so, tcn)

                def mlp_head(t, h_cur):
                    for m in range(H2 // 128):
                        ps = psm.tile([128, BS], F32, tag="m1")
                        for k in range(KH):
                            nc.tensor.matmul(
                                ps, w1_sb[:, k, m * 128 : (m + 1) * 128],
                                h_cur[:, k, :],
                                start=(k == 0), stop=(k == KH - 1),
                            )
                        nc.scalar.activation(
                            out=relu1T[:, m, :], in_=ps, func=AF.Relu,
                            bias=b1_sb[:, m : m + 1], scale=1.0,
                        )
                    for b in range(NB):
                        ps2 = psw.tile([128, A], F32, tag="w2")
                        for k2 in range(H2 // 128):
                            nc.tensor.matmul(
                                ps2,
                                relu1T[:, k2, b * 128 : (b + 1) * 128],
                                w2_sb[:, k2, :],
                                start=(k2 == 0), stop=(k2 == H2 // 128 - 1),
                            )
                        t6 = ac.tile([128, A], F32, tag="t6")
                        nc.vector.tensor_add(t6, ps2, b2_b)
                        nc.scalar.activation(
                            out=mu_sb[:, b, t * A : (t + 1) * A], in_=t6, func=AF.Tanh
                        )

                # ---- step 0: h0 = c0 = 0 -> gates = pre ----
                h_new = hT[0]
                for j in range(KH):
                    si = ac.tile([128, BS], F32, tag="a0")
                    tg = ac.tile([128, BS], F32, tag="a2")
                    so = ac.tile([128, BS], F32, tag="a3")
                    nc.scalar.activation(out=si, in_=preT[:, j, :], func=AF.Sigmoid)
                    nc.scalar.activation(out=tg, in_=preT[:, 16 + j, :], func=AF.Tanh)
                    nc.scalar.activation(out=so, in_=preT[:, 24 + j, :], func=AF.Sigmoid)
                    cell_update(j, si, None, tg, so, first=True)
                mlp_head(0, hT[0])

                # ---- steps 1..15 ----
                for t in range(1, T):
                    h_cur = hT[(t + 1) % 2]
                    h_new = hT[t % 2]
                    for j in range(KH):
                        acts = {}
                        for q in range(4):
                            m = 8 * q + j
                            ps = psg.tile([128, BS], F32, tag="g")
                            for kk in range(KH):
                                k = (kk + j) % KH
                                nc.tensor.matmul(
                                    ps,
                                    whh_sb[:, k, m * 128 : (m + 1) * 128],
                                    h_cur[:, k, :],
                                    start=(kk == 0), stop=(kk == KH - 1),
                                )
                            tmp = ac.tile([128, BS], F32, tag=f"q{q}")
                            nc.vector.tensor_add(tmp, ps, preT[:, m, :])
                            out_a = ac.tile([128, BS], F32, tag=f"a{q}")
                            nc.scalar.activation(
                                out=out_a, in_=tmp,
                                func=AF.Tanh if q == 2 else AF.Sigmoid,
                            )
                            acts[q] = out_a
                        cell_update(j, acts[0], acts[1], acts[2], acts[3], first=False)
                    mlp_head(t, h_new)

            # ---- write out ----
            nc.sync.dma_start(
                mu_d.ap().rearrange("(bt p) f -> p bt f", p=128), mu_sb
            )

    nc.compile()
    return nc


def kernel(**inputs):
    obs = np.asarray(inputs["obs"], np.float32)
    W_trunk = np.asarray(inputs["W_trunk"], np.float32)
    b_trunk = np.asarray(inputs["b_trunk"], np.float32)
    gamma = np.asarray(inputs["gamma"], np.float32)
    beta = np.asarray(inputs["beta"], np.float32)
    W_ih = np.asarray(inputs["W_ih"], np.float32)
    b_ih = np.asarray(inputs["b_ih"], np.float32)
    W_hh = np.asarray(inputs["W_hh"], np.float32)
    b_hh = np.asarray(inputs["b_hh"], np.float32)
    W1 = np.asarray(inputs["W1"], np.float32)
    b1 = np.asarray(inputs["b1"], np.float32)
    W2 = np.asarray(inputs["W2"], np.float32)
    b2 = np.asarray(inputs["b2"], np.float32)
    num_actions = int(np.asarray(inputs["num_actions"]))
    assert num_actions == T, f"kernel hardcodes T={T}, got {num_actions}"
    assert obs.shape == (B, R)

    if "nc" not in _CACHE:
        _CACHE["nc"] = _build()
    nc = _CACHE["nc"]

    wtr = np.zeros((RP, Fd), BF)
    wtr[:R] = W_trunk.astype(BF)
    wih = W_ih.astype(BF)
    whh = W_hh.astype(BF)
    w1 = W1.astype(BF)
    w2 = W2.astype(BF)
    bsum = (b_ih + b_hh).astype(np.float32)

    in_maps = []
    for i in range(NC_):
        sh = obs[i * BS : (i + 1) * BS]           # [256, R]
        obsT = np.zeros((RP, BS), BF)
        obsT[:R] = np.ascontiguousarray(sh.T).astype(BF)
        in_maps.append({
            "obsT": obsT, "wtr": wtr, "wih": wih, "whh": whh,
            "w1": w1, "w2": w2, "btr": b_trunk, "gam": gamma,
            "bet": beta, "bsum": bsum, "b1": b1, "b2": b2,
        })

    res = bass_utils.run_bass_kernel_spmd(
        nc, in_maps, core_ids=list(range(NC_)),
        trace=bool(int(__import__("os").environ.get("KTRACE", "0"))),
    )
    _CACHE["last_result"] = res
    out = np.concatenate(
        [res.results[i]["mu"].reshape(BS, T, A) for i in range(NC_)], axis=0
    )
    return out


# revision 8
# speedup vs baseline: 1.4041x; 1.0193x over previous
"""Trainium2 Bass kernel for nn_LSTMActor: trunk GEMM -> LayerNorm -> Tanh ->
LSTM (16 steps, constant input) -> MLP head -> tanh.

Sharding: data-parallel over batch B=2048 across 8 cores (256 rows each);
all weights replicated. Everything after the trunk runs in a transposed
layout (feature dim on partitions) so no per-step transposes are needed.
"""

import numpy as np
import ml_dtypes

import concourse.bass as bass
import concourse.tile as tile
from concourse import mybir, bacc
from concourse import bass_utils
from concourse.masks import make_identity

BF = ml_dtypes.bfloat16
F32 = mybir.dt.float32
BF16 = mybir.dt.bfloat16

B, R, Fd, H, A, T = 2048, 39200, 1024, 1024, 6, 16
NC_ = 8
BS = B // NC_          # 256 rows per core
NB = BS // 128         # 2 b-tiles per core
KT = 128               # contraction tile
RP = ((R + KT - 1) // KT) * KT   # 39296, padded R
NK = RP // KT          # 307 K-tiles for trunk
KH = H // 128          # 8 K-tiles for H-dim GEMMs
M4 = 4 * H // 128      # 32 M-tiles of gates
H2 = H // 2            # 512
KG = 2                 # trunk K-tiles per DMA batch (256KB wtr + 64KB obsT)

_CACHE = {}


def _build():
    nc = bacc.Bacc("TRN2", target_bir_lowering=False, debug=False)

    obsT_d = nc.dram_tensor("obsT", [RP, BS], BF16, kind="ExternalInput")
    wtr_d = nc.dram_tensor("wtr", [RP, Fd], BF16, kind="ExternalInput")
    wih_d = nc.dram_tensor("wih", [M4, 128, KH * 128], BF16, kind="ExternalInput")
    whh_d = nc.dram_tensor("whh", [H, 4 * H], BF16, kind="ExternalInput")
    w1_d = nc.dram_tensor("w1", [H, H2], BF16, kind="ExternalInput")
    w2_d = nc.dram_tensor("w2", [H2, A], BF16, kind="ExternalInput")
    btr_d = nc.dram_tensor("btr", [Fd], F32, kind="ExternalInput")
    gam_d = nc.dram_tensor("gam", [Fd], F32, kind="ExternalInput")
    bet_d = nc.dram_tensor("bet", [Fd], F32, kind="ExternalInput")
    bsum_d = nc.dram_tensor("bsum", [4 * H], F32, kind="ExternalInput")
    b1_d = nc.dram_tensor("b1", [H2], F32, kind="ExternalInput")
    b2_d = nc.dram_tensor("b2", [A], F32, kind="ExternalInput")
    mu_d = nc.dram_tensor("mu", [BS, T * A], F32, kind="ExternalOutput")

    AF = mybir.ActivationFunctionType

    def bc(ap1d, p=128):
        return bass.AP(tensor=ap1d.tensor, offset=ap1d.offset,
                       ap=[[0, p]] + [list(x) for x in ap1d.ap])

    with tile.TileContext(nc) as tc:
        with (
            tc.tile_pool(name="const", bufs=1) as cst,
            tc.tile_pool(name="state", bufs=1) as st,
            tc.tile_pool(name="wstream", bufs=2) as ws,
            tc.tile_pool(name="work", bufs=1) as wk,
            tc.tile_pool(name="acts", bufs=2) as ac,
        ):
            # ---- resident constants ----
            ident = cst.tile([128, 128], BF16)
            make_identity(nc, ident)
            whh_sb = cst.tile([128, KH, 4 * H], BF16)     # 64KB/part
            w1_sb = cst.tile([128, KH, H2], BF16)         # 8KB/part
            w2_sb = cst.tile([128, H2 // 128, A], BF16)   # [128,4,6]
            btr_b = cst.tile([128, Fd], F32)
            gam_b = cst.tile([128, Fd], F32)
            bet_b = cst.tile([128, Fd], F32)
            bsum_sb = cst.tile([128, M4], F32)            # [128,32] col m = bsum[m*128+p]
            nc.sync.dma_start(bsum_sb, bsum_d.ap().rearrange("(m p) -> p m", p=128))
            b1_sb = cst.tile([128, H2 // 128], F32)       # [128,4]
            nc.sync.dma_start(b1_sb, b1_d.ap().rearrange("(m p) -> p m", p=128))
            b2_b = cst.tile([128, A], F32)
            nc.sync.dma_start(b2_b, bc(b2_d.ap()))
            eps_t = cst.tile([128, 1], F32)
            nc.vector.memset(eps_t, 1e-5)

            # ---- persistent state ----
            xT = st.tile([128, KH, BS], BF16)             # x^T  [Fd, BS]
            preT = st.tile([128, M4, BS], BF16)            # pre^T [4H, BS] 32KB/part
            c_st = st.tile([128, KH, BS], F32)            # c^T  [H, BS]
            hT = [st.tile([128, KH, BS], BF16, name=f"hT{i}", tag=f"h{i}") for i in range(2)]  # ping-pong
            mu_sb = st.tile([128, NB, T * A], F32)        # [128,2,96]

            wtr_r = wtr_d.ap().rearrange("(ko p) n -> p ko n", p=128)
            obsT_r = obsT_d.ap().rearrange("(ko p) b -> p ko b", p=128)

            # ================= Phase 1: trunk GEMM =================
            with tc.tile_pool(name="ps_trunk", bufs=1, space="PSUM") as pst:
                psx = pst.tile([128, NB, Fd], F32)        # 8KB/part = 4 banks
                for kg in range(0, NK, KG):
                    kn = min(KG, NK - kg)
                    wt = ws.tile([128, KG, Fd], BF16, tag="wtr", bufs=6)
                    ot = ws.tile([128, KG, BS], BF16, tag="obsT", bufs=4)
                    nc.sync.dma_start(wt[:, :kn, :], wtr_r[:, kg : kg + kn, :])
                    nc.sync.dma_start(ot[:, :kn, :], obsT_r[:, kg : kg + kn, :])
                    for kk in range(kn):
                        k = kg + kk
                        for b in range(NB):
                            lhsT = ot[:, kk, b * 128 : (b + 1) * 128]
                            for n in range(2):
                                nc.tensor.matmul(
                                    psx[:, b, n * 512 : (n + 1) * 512],
                                    lhsT,
                                    wt[:, kk, n * 512 : (n + 1) * 512],
                                    start=(k == 0),
                                    stop=(k == NK - 1),
                                )

                # LN constants: needed right after trunk; emitted here so the
                # trunk's first chunks aren't queued behind them
                nc.sync.dma_start(btr_b, bc(btr_d.ap()))
                nc.sync.dma_start(gam_b, bc(gam_d.ap()))
                nc.sync.dma_start(bet_b, bc(bet_d.ap()))

                # ============ Phase 2: LayerNorm + tanh ============
                xa = wk.tile([128, NB, Fd], BF16, tag="xa")
                for b in range(NB):
                    xs = wk.tile([128, Fd], F32, tag="xs", bufs=2)
                    nc.vector.tensor_add(xs, psx[:, b, :], btr_b)
                    stats = wk.tile([128, 2, 6], F32, tag="stats")
                    for s in range(2):
                        nc.vector.bn_stats(
                            out=stats[:, s, :], in_=xs[:, s * 512 : (s + 1) * 512]
                        )
                    mv = wk.tile([128, 2], F32, tag="mv")
                    nc.vector.bn_aggr(out=mv, in_=stats)
                    rstd = wk.tile([128, 1], F32, tag="rstd")
                    nc.scalar.activation(
                        out=rstd, in_=mv[:, 1:2], func=AF.Sqrt, bias=eps_t, scale=1.0
                    )
                    nc.vector.reciprocal(out=rstd, in_=rstd)
                    nc.vector.scalar_tensor_tensor(
                        out=xs, in0=xs, scalar=mv[:, 0:1], in1=gam_b,
                        op0=mybir.AluOpType.subtract, op1=mybir.AluOpType.mult,
                    )
                    nc.vector.scalar_tensor_tensor(
                        out=xs, in0=xs, scalar=rstd, in1=bet_b,
                        op0=mybir.AluOpType.mult, op1=mybir.AluOpType.add,
                    )
                    nc.scalar.activation(out=xa[:, b, :], in_=xs, func=AF.Tanh)

            # ============ Phase 3: transpose x -> xT (bf16) ============
            with tc.tile_pool(name="ps_tr", bufs=4, space="PSUM") as ptr:
                for b in range(NB):
                    for f in range(KH):
                        pt = ptr.tile([128, 128], BF16, tag="tr")
                        nc.tensor.transpose(
                            pt, xa[:, b, f * 128 : (f + 1) * 128], ident
                        )
                        nc.scalar.activation(
                            out=xT[:, f, b * 128 : (b + 1) * 128], in_=pt, func=AF.Copy
                        )

            # ============ Phase 4: pre^T = W_ih^T x^T + bsum ============
            with tc.tile_pool(name="ps_pre", bufs=2, space="PSUM") as ppr:
                whh_r = whh_d.ap().rearrange("(ko p) n -> p ko n", p=128)
                for m in range(M4):
                    wm = ws.tile([128, KH, 128], BF16, tag="wih", bufs=4)
                    nc.sync.dma_start(
                        wm, wih_d.ap()[m].rearrange("p (k j) -> p k j", j=128))
                    if m % 4 == 0:
                        k8 = m // 4
                        nc.sync.dma_start(whh_sb[:, k8, :], whh_r[:, k8, :])
                        if k8 == 0:
                            nc.sync.dma_start(
                                w1_sb, w1_d.ap().rearrange("(ko p) n -> p ko n", p=128))
                            nc.sync.dma_start(
                                w2_sb, w2_d.ap().rearrange("(ko p) n -> p ko n", p=128))
                    ps = ppr.tile([128, BS], F32, tag="pre")
                    for k in range(KH):
                        nc.tensor.matmul(
                            ps, wm[:, k, :], xT[:, k, :],
                            start=(k == 0), stop=(k == KH - 1),
                        )
                    nc.vector.tensor_scalar_add(
                        preT[:, m, :], ps, bsum_sb[:, m : m + 1]
                    )

            # ============ Phase 5: LSTM steps ============
            with (
                tc.tile_pool(name="ps_g", bufs=5, space="PSUM") as psg,
                tc.tile_pool(name="ps_m", bufs=2, space="PSUM") as psm,
                tc.tile_pool(name="ps_w2", bufs=1, space="PSUM") as psw,
            ):
                relu1T = st.tile([128, H2 // 128, BS], BF16)

                def cell_update(j, si, sf, tg, so, first):
                    """c[j] = sf*c[j] + si*tg ; h[j] = so*tanh(c[j]) -> h_new."""
                    if first:
                        nc.vector.tensor_mul(c_st[:, j, :], si, tg)
                    else:
                        t1 = ac.tile([128, BS], F32, tag="t1")
                        nc.vector.tensor_mul(t1, si, tg)
                        nc.vector.tensor_mul(c_st[:, j, :], c_st[:, j, :], sf)
                        nc.vector.tensor_add(c_st[:, j, :], c_st[:, j, :], t1)
                    tcn = ac.tile([128, BS], F32, tag="tc")
                    nc.scalar.activation(out=tcn, in_=c_st[:, j, :], func=AF.Tanh)
                    nc.vector.tensor_mul(h_new[:, j, :], so, tcn)

                def mlp_head(t, h_cur):
                    for m in range(H2 // 128):
                        ps = psm.tile([128, BS], F32, tag="m1")
                        for k in range(KH):
                            nc.tensor.matmul(
                                ps, w1_sb[:, k, m * 128 : (m + 1) * 128],
                                h_cur[:, k, :],
                                start=(k == 0), stop=(k == KH - 1),
                            )
                        nc.scalar.activation(
                            out=relu1T[:, m, :], in_=ps, func=AF.Relu,
                            bias=b1_sb[:, m : m + 1], scale=1.0,
                        )
                    for b in range(NB):
                        ps2 = psw.tile([128, A], F32, tag="w2")
                        for k2 in range(H2 // 128):
                            nc.tensor.matmul(
                                ps2,
                                relu1T[:, k2, b * 128 : (b + 1) * 128],
                                w2_sb[:, k2, :],
                                start=(k2 == 0), stop=(k2 == H2 // 128 - 1),
                            )
                        t6 = ac.tile([128, A], F32, tag="t6")
                        nc.vector.tensor_add(t6, ps2, b2_b)
                        nc.scalar.activation(
                            out=mu_sb[:, b, t * A : (t + 1) * A], in_=t6, func=AF.Tanh
                        )

                # ---- step 0: h0 = c0 = 0 -> gates = pre ----
                h_new = hT[0]
                for j in range(KH):
                    si = ac.tile([128, BS], F32, tag="a0")
                    tg = ac.tile([128, BS], F32, tag="a2")
                    so = ac.tile([128, BS], F32, tag="a3")
                    nc.scalar.activation(out=si, in_=preT[:, j, :], func=AF.Sigmoid)
                    nc.scalar.activation(out=tg, in_=preT[:, 16 + j, :], func=AF.Tanh)
                    nc.scalar.activation(out=so, in_=preT[:, 24 + j, :], func=AF.Sigmoid)
                    cell_update(j, si, None, tg, so, first=True)
                mlp_head(0, hT[0])

                # ---- steps 1..15 ----
                for t in range(1, T):
                    h_cur = hT[(t + 1) % 2]
                    h_new = hT[t % 2]
                    for j in range(KH):
                        acts = {}
                        for q in range(4):
                            m = 8 * q + j
                            ps = psg.tile([128, BS], F32, tag="g")
                            for kk in range(KH):
                                k = (kk + j) % KH
                                nc.tensor.matmul(
                                    ps,
                                    whh_sb[:, k, m * 128 : (m + 1) * 128],
                                    h_cur[:, k, :],
                                    start=(kk == 0), stop=(kk == KH - 1),
                                )
                            tmp = ac.tile([128, BS], F32, tag=f"q{q}")
                            nc.vector.tensor_add(tmp, ps, preT[:, m, :])
                            out_a = ac.tile([128, BS], F32, tag=f"a{q}")
                            nc.scalar.activation(
                                out=out_a, in_=tmp,
                                func=AF.Tanh if q == 2 else AF.Sigmoid,
                            )
                            acts[q] = out_a
                        cell_update(j, acts[0], acts[1], acts[2], acts[3], first=False)
                    mlp_head(t, h_new)

            # ---- write out ----
            nc.sync.dma_start(
                mu_d.ap().rearrange("(bt p) f -> p bt f", p=128), mu_sb
            )

    nc.compile()
    return nc


def kernel(**inputs):
    obs = np.asarray(inputs["obs"], np.float32)
    W_trunk = np.asarray(inputs["W_trunk"], np.float32)
    b_trunk = np.asarray(inputs["b_trunk"], np.float32)
    gamma = np.asarray(inputs["gamma"], np.float32)
    beta = np.asarray(inputs["beta"], np.float32)
    W_ih = np.asarray(inputs["W_ih"], np.float32)
    b_ih = np.asarray(inputs["b_ih"], np.float32)
    W_hh = np.asarray(inputs["W_hh"], np.float32)
    b_hh = np.asarray(inputs["b_hh"], np.float32)
    W1 = np.asarray(inputs["W1"], np.float32)
    b1 = np.asarray(inputs["b1"], np.float32)
    W2 = np.asarray(inputs["W2"], np.float32)
    b2 = np.asarray(inputs["b2"], np.float32)
    num_actions = int(np.asarray(inputs["num_actions"]))
    assert num_actions == T, f"kernel hardcodes T={T}, got {num_actions}"
    assert obs.shape == (B, R)

    if "nc" not in _CACHE:
        _CACHE["nc"] = _build()
    nc = _CACHE["nc"]

    wtr = np.zeros((RP, Fd), BF)
    wtr[:R] = W_trunk.astype(BF)
    wih = np.ascontiguousarray(
        W_ih.astype(BF).reshape(KH, 128, M4, 128).transpose(2, 1, 0, 3)
    ).reshape(M4, 128, KH * 128)
    whh = W_hh.astype(BF)
    w1 = W1.astype(BF)
    w2 = W2.astype(BF)
    bsum = (b_ih + b_hh).astype(np.float32)

    in_maps = []
    for i in range(NC_):
        sh = obs[i * BS : (i + 1) * BS]           # [256, R]
        obsT = np.zeros((RP, BS), BF)
        obsT[:R] = np.ascontiguousarray(sh.T).astype(BF)
        in_maps.append({
            "obsT": obsT, "wtr": wtr, "wih": wih, "whh": whh,
            "w1": w1, "w2": w2, "btr": b_trunk, "gam": gamma,
            "bet": beta, "bsum": bsum, "b1": b1, "b2": b2,
        })

    res = bass_utils.run_bass_kernel_spmd(
        nc, in_maps, core_ids=list(range(NC_)),
        trace=bool(int(__import__("os").environ.get("KTRACE", "0"))),
    )
    _CACHE["last_result"] = res
    out = np.concatenate(
        [res.results[i]["mu"].reshape(BS, T, A) for i in range(NC_)], axis=0
    )
    return out
